# revision 4
# baseline (speedup 1.0000x reference)
"""GCN encoder (3-layer: GCNConv+BN+ReLU ×2, then GCNConv to [mu|logvar]) on 8
Trainium2 NeuronCores via Bass/Tile.

Strategy (dst-sharded message passing):
- Nodes are sharded 8 ways by destination. Each core owns a contiguous dst
  range and processes only the edges pointing into it (plus self-loops,
  appended as ordinary edges with coef = dinv^2).
- Host preprocessing packs each core's edges into dst-blocks (<=128 dst nodes,
  <= TPB*128 edge slots each, zero-padded) so one static program serves all 8
  cores. Node ids are remapped into a padded global layout so gather tables
  and output shards line up with block boundaries.
- Per edge-tile (128 edges): a 128-row indirect DMA gathers src feature rows
  from the (replicated) table; DVE builds a one-hot selection matrix weighted
  by the edge coefficients (iota compare x coef); TensorE accumulates
  sel^T-weighted rows into a feature-major PSUM block: agg[feat, dst].
- Per block: the layer weight matmul runs on the aggregated block (linearity:
  S@(hW) == (S@h)@W), BN partial sums accumulate on DVE.
- BatchNorm: [128,2] AllReduce of (sum, sumsq) across cores; scale/shift and
  ReLU are fused into one ScalarE activation over the feature-major shard.
- Halo exchange: shards are PE-transposed to row-major and AllGathered into
  each core's next-layer gather table.
- Layer 3 uses W_cat = [W_mu | W_logvar] so both outputs ride one aggregation.
"""

import hashlib
import numpy as np

N = 50000
E = 800000
D = 128
DLAT = 64
EPS = 1e-5
NCORES = 8
NSHARD = N // NCORES          # 6250
TPB = 18                      # edge tiles per dst block (18*128 = 2304 slots)

_CACHE = {}
_NCONV = 3        # debug: number of convs to run
_USE_AR = True    # debug: BN AllReduce on/off (off -> local stats)
_USE_AG = True    # debug: AllGather on/off (off -> next conv gathers garbage)


# ----------------------------------------------------------------------------
# Host-side preprocessing
# ----------------------------------------------------------------------------

def _preprocess(edge_index):
    src = np.asarray(edge_index[0], dtype=np.int64)
    dst = np.asarray(edge_index[1], dtype=np.int64)
    deg = np.bincount(dst, minlength=N).astype(np.float32) + 1.0
    dinv = (1.0 / np.sqrt(deg)).astype(np.float32)

    allv = np.arange(N, dtype=np.int64)
    src_a = np.concatenate([src, allv])
    dst_a = np.concatenate([dst, allv])
    coef_a = dinv[src_a] * dinv[dst_a]

    cap = TPB * 128
    per_core = []
    max_blocks = 0
    for c in range(NCORES):
        lo, hi = c * NSHARD, (c + 1) * NSHARD
        m = (dst_a >= lo) & (dst_a < hi)
        s_c, d_c, f_c = src_a[m], dst_a[m] - lo, coef_a[m]
        order = np.argsort(d_c, kind="stable")
        s_c, d_c, f_c = s_c[order], d_c[order], f_c[order]
        cnt = np.bincount(d_c, minlength=NSHARD)
        # greedy block packing: <=128 dsts and <=cap edges per block
        blocks = []  # (dst_start, dst_end, edge_start, edge_end)
        v = 0
        epos = 0
        while v < NSHARD:
            v0, e0, ecnt = v, epos, 0
            while v < NSHARD and (v - v0) < 128 and ecnt + cnt[v] <= cap:
                ecnt += cnt[v]
                v += 1
            assert v > v0, f"node {v} degree {cnt[v]} exceeds capacity {cap}"
            epos += ecnt
            blocks.append((v0, v, e0, epos))
        per_core.append((s_c, d_c, f_c, blocks))
        max_blocks = max(max_blocks, len(blocks))

    B = max_blocks
    padn = NCORES * B * 128

    # padded global position of each node
    ppos = np.zeros(N, dtype=np.int64)
    for c in range(NCORES):
        _, _, _, blocks = per_core[c]
        for b, (v0, v1, _, _) in enumerate(blocks):
            ppos[c * NSHARD + v0:c * NSHARD + v1] = (
                c * B * 128 + b * 128 + np.arange(v1 - v0)
            )

    # packed per-core device arrays
    srci = np.zeros((NCORES, 128, B * TPB), dtype=np.int32)
    dstf = np.zeros((NCORES, 128, B * TPB), dtype=np.float32)
    coww = np.zeros((NCORES, 128, B * TPB), dtype=np.float32)
    for c in range(NCORES):
        s_c, d_c, f_c, blocks = per_core[c]
        sp = np.zeros(B * cap, dtype=np.int64)
        dp = np.zeros(B * cap, dtype=np.float32)
        fp = np.zeros(B * cap, dtype=np.float32)
        for b, (v0, v1, e0, e1) in enumerate(blocks):
            n = e1 - e0
            sp[b * cap:b * cap + n] = ppos[s_c[e0:e1]]
            dp[b * cap:b * cap + n] = (d_c[e0:e1] - v0).astype(np.float32)
            fp[b * cap:b * cap + n] = f_c[e0:e1]
        # wrap: [B*TPB tiles, 128] -> [128, B*TPB]; tile k edge p at [p, k]
        srci[c] = sp.reshape(B * TPB, 128).T
        dstf[c] = dp.reshape(B * TPB, 128).T
        coww[c] = fp.reshape(B * TPB, 128).T

    return B, padn, ppos, srci, dstf, coww


# ----------------------------------------------------------------------------
# Device program
# ----------------------------------------------------------------------------

def _build(B, padn):
    from concourse import bacc, mybir
    import concourse.bass as bass
    import concourse.tile as tile
    from concourse.masks import make_identity

    F32 = mybir.dt.float32
    NB = B * 128  # padded shard width

    nc = bacc.Bacc("TRN2", target_bir_lowering=False, debug=False,
                   num_devices=NCORES)
    xtab = nc.dram_tensor("xtab", [padn, D], F32, kind="ExternalInput").ap()
    srci = nc.dram_tensor("srci", [128, B * TPB], mybir.dt.int32, kind="ExternalInput").ap()
    dstf = nc.dram_tensor("dstf", [128, B * TPB], F32, kind="ExternalInput").ap()
    coef = nc.dram_tensor("coef", [128, B * TPB], F32, kind="ExternalInput").ap()
    iotab = nc.dram_tensor("iotab", [128, TPB * 128], F32, kind="ExternalInput").ap()
    w0 = nc.dram_tensor("w0", [D, D], F32, kind="ExternalInput").ap()
    w1 = nc.dram_tensor("w1", [D, D], F32, kind="ExternalInput").ap()
    wcat = nc.dram_tensor("wcat", [D, D], F32, kind="ExternalInput").ap()
    gb0 = nc.dram_tensor("gb0", [128, 2], F32, kind="ExternalInput").ap()  # gamma0|beta0
    gb1 = nc.dram_tensor("gb1", [128, 2], F32, kind="ExternalInput").ap()
    zout = nc.dram_tensor("z", [NB, D], F32, kind="ExternalOutput").ap()

    htab1 = nc.dram_tensor("htab1", [padn, D], F32)
    htab2 = nc.dram_tensor("htab2", [padn, D], F32)
    hrm1 = nc.dram_tensor("hrm1", [NB, D], F32)
    hrm2 = nc.dram_tensor("hrm2", [NB, D], F32)
    bnin = [nc.dram_tensor(f"bnin{k}", [128, 2], F32) for k in range(2)]
    bnout = [nc.dram_tensor(f"bnout{k}", [128, 2], F32) for k in range(2)]

    with tile.TileContext(nc) as tc:
        with (
            tc.tile_pool(name="const", bufs=1) as constp,
            tc.tile_pool(name="gath", bufs=12) as gpool,
            tc.tile_pool(name="selp", bufs=3) as selp,
            tc.tile_pool(name="work", bufs=4) as wpool,
            tc.tile_pool(name="hacc", bufs=1) as haccp,
            tc.tile_pool(name="psA", bufs=2, space="PSUM") as psA,
            tc.tile_pool(name="psB", bufs=2, space="PSUM") as psB,
            tc.tile_pool(name="psT", bufs=2, space="PSUM") as psT,
        ):
            idx_t = constp.tile([128, B * TPB], mybir.dt.int32)
            dst_t = constp.tile([128, B * TPB], F32)
            cof_t = constp.tile([128, B * TPB], F32)
            iota_t = constp.tile([128, TPB, 128], F32)
            ident = constp.tile([128, 128], F32)
            w_t = [constp.tile([D, D], F32, name=f"w_t{k}", tag=f"w{k}") for k in range(3)]
            gb_t = [constp.tile([128, 2], F32, name=f"gb_t{k}", tag=f"gb{k}") for k in range(2)]
            nc.sync.dma_start(out=idx_t[:], in_=srci[:])
            nc.sync.dma_start(out=dst_t[:], in_=dstf[:])
            nc.sync.dma_start(out=cof_t[:], in_=coef[:])
            nc.sync.dma_start(out=iota_t[:], in_=iotab[:].rearrange("p (t f) -> p t f", t=TPB))
            nc.sync.dma_start(out=w_t[0][:], in_=w0[:])
            nc.sync.dma_start(out=w_t[1][:], in_=w1[:])
            nc.sync.dma_start(out=w_t[2][:], in_=wcat[:])
            nc.sync.dma_start(out=gb_t[0][:], in_=gb0[:])
            nc.sync.dma_start(out=gb_t[1][:], in_=gb1[:])
            make_identity(nc, ident[:])

            h_shard = haccp.tile([128, NB], F32)
            ssum = haccp.tile([128, 1], F32)
            ssq = haccp.tile([128, 1], F32)

            tabs = [xtab, htab1.ap(), htab2.ap()]
            hrms = [hrm1, hrm2]
            htabs = [htab1, htab2]

            for k in range(_NCONV):
                tab = tabs[k]
                if k < 2:
                    nc.vector.memset(ssum[:], 0.0)
                    nc.vector.memset(ssq[:], 0.0)
                for b in range(B):
                    # one-hot (coef-weighted) selection for the whole block
                    sel = selp.tile([128, TPB, 128], F32)
                    c0 = b * TPB
                    nc.vector.tensor_tensor(
                        out=sel[:],
                        in0=dst_t[:, c0:c0 + TPB].to_broadcast([128, TPB, 128]),
                        in1=iota_t[:], op=mybir.AluOpType.is_equal,
                    )
                    nc.vector.tensor_tensor(
                        out=sel[:], in0=sel[:],
                        in1=cof_t[:, c0:c0 + TPB].to_broadcast([128, TPB, 128]),
                        op=mybir.AluOpType.mult,
                    )
                    agg_ps = psA.tile([128, 128], F32, space="PSUM")
                    for t in range(TPB):
                        kk = c0 + t
                        g = gpool.tile([128, D], F32)
                        nc.gpsimd.indirect_dma_start(
                            out=g[:], out_offset=None, in_=tab,
                            in_offset=bass.IndirectOffsetOnAxis(
                                ap=idx_t[:, kk:kk + 1], axis=0),
                        )
                        # agg[feat, dst] += g[e, feat]^T @ sel[e, dst]
                        nc.tensor.matmul(out=agg_ps[:], lhsT=g[:],
                                         rhs=sel[:, t, :],
                                         start=(t == 0), stop=(t == TPB - 1))
                    aggs = wpool.tile([128, 128], F32, tag="aggs")
                    nc.vector.tensor_copy(out=aggs[:], in_=agg_ps[:])
                    # h[fo, dst] = W[fi, fo]^T @ aggs[fi, dst]
                    h_ps = psB.tile([128, 128], F32, space="PSUM")
                    nc.tensor.matmul(out=h_ps[:], lhsT=w_t[k][:], rhs=aggs[:],
                                     start=True, stop=True)
                    h_sl = h_shard[:, b * 128:(b + 1) * 128]
                    nc.vector.tensor_copy(out=h_sl, in_=h_ps[:])
                    if k < 2:
                        rsum = wpool.tile([128, 1], F32, tag="rsum")
                        nc.vector.tensor_reduce(out=rsum[:], in_=h_ps[:],
                                                axis=mybir.AxisListType.X,
                                                op=mybir.AluOpType.add)
                        nc.vector.tensor_add(out=ssum[:], in0=ssum[:], in1=rsum[:])
                        sq = wpool.tile([128, 128], F32, tag="sq")
                        nc.vector.tensor_tensor(out=sq[:], in0=h_sl, in1=h_sl,
                                                op=mybir.AluOpType.mult)
                        nc.vector.tensor_reduce(out=rsum[:], in_=sq[:],
                                                axis=mybir.AxisListType.X,
                                                op=mybir.AluOpType.add)
                        nc.vector.tensor_add(out=ssq[:], in0=ssq[:], in1=rsum[:])

                if k < 2:
                    # cross-core BN stats
                    stl = wpool.tile([128, 2], F32, tag="stl")
                    nc.vector.tensor_copy(out=stl[:, 0:1], in_=ssum[:])
                    nc.vector.tensor_copy(out=stl[:, 1:2], in_=ssq[:])
                    nc.sync.dma_start(out=bnin[k].ap(), in_=stl[:])
                    stg = wpool.tile([128, 2], F32, tag="stg")
                    if _USE_AR:
                        nc.gpsimd.collective_compute(
                            "AllReduce", mybir.AluOpType.add,
                            replica_groups=[list(range(NCORES))],
                            ins=[bnin[k].ap().opt()], outs=[bnout[k].ap().opt()],
                        )
                        nc.sync.dma_start(out=stg[:], in_=bnout[k].ap())
                    else:
                        nc.sync.dma_start(out=stg[:], in_=bnin[k].ap())
                    # a = gamma*rsqrt(var+eps); b = beta - mean*a
                    mean = wpool.tile([128, 1], F32, tag="mean")
                    var = wpool.tile([128, 1], F32, tag="var")
                    a_t = wpool.tile([128, 1], F32, tag="a_t")
                    b_t = wpool.tile([128, 1], F32, tag="b_t")
                    inv_n = 1.0 / float(N)
                    nc.vector.tensor_scalar_mul(out=mean[:], in0=stg[:, 0:1], scalar1=inv_n)
                    nc.vector.tensor_scalar_mul(out=var[:], in0=stg[:, 1:2], scalar1=inv_n)
                    msq = wpool.tile([128, 1], F32, tag="msq")
                    nc.vector.tensor_tensor(out=msq[:], in0=mean[:], in1=mean[:],
                                            op=mybir.AluOpType.mult)
                    nc.vector.tensor_tensor(out=var[:], in0=var[:], in1=msq[:],
                                            op=mybir.AluOpType.subtract)
                    nc.vector.tensor_scalar(out=var[:], in0=var[:], scalar1=float(EPS),
                                            scalar2=None, op0=mybir.AluOpType.add)
                    nc.scalar.activation(out=a_t[:], in_=var[:],
                                         func=mybir.ActivationFunctionType.Sqrt)
                    nc.vector.reciprocal(out=a_t[:], in_=a_t[:])
                    nc.vector.tensor_tensor(out=a_t[:], in0=a_t[:],
                                            in1=gb_t[k][:, 0:1],
                                            op=mybir.AluOpType.mult)
                    nc.vector.tensor_tensor(out=b_t[:], in0=mean[:], in1=a_t[:],
                                            op=mybir.AluOpType.mult)
                    nc.vector.tensor_tensor(out=b_t[:], in0=gb_t[k][:, 1:2],
                                            in1=b_t[:],
                                            op=mybir.AluOpType.subtract)
                    # h = relu(a*h + b), fused on ScalarE
                    nc.scalar.activation(out=h_shard[:], in_=h_shard[:],
                                         func=mybir.ActivationFunctionType.Relu,
                                         scale=a_t[:], bias=b_t[:])

                # export shard row-major (+ halo exchange for k<2)
                is_last = (k == _NCONV - 1)
                dst_dram = None if is_last else hrms[k]
                for b in range(B):
                    tp = psT.tile([128, 128], F32, space="PSUM")
                    nc.tensor.transpose(out=tp[:],
                                        in_=h_shard[:, b * 128:(b + 1) * 128],
                                        identity=ident[:])
                    rm = wpool.tile([128, 128], F32, tag="rm")
                    nc.vector.tensor_copy(out=rm[:], in_=tp[:])
                    if not is_last:
                        nc.sync.dma_start(out=dst_dram.ap()[b * 128:(b + 1) * 128, :],
                                          in_=rm[:])
                    else:
                        nc.sync.dma_start(out=zout[b * 128:(b + 1) * 128, :],
                                          in_=rm[:])
                if k < 2 and k < _NCONV - 1:
                    if _USE_AG:
                        nc.gpsimd.collective_compute(
                            "AllGather", mybir.AluOpType.bypass,
                            replica_groups=[list(range(NCORES))],
                            ins=[hrms[k].ap().opt()], outs=[htabs[k].ap().opt()],
                        )
                    else:
                        nc.sync.dma_start(out=htabs[k].ap()[0:NB, :], in_=hrms[k].ap())

    nc.compile()
    return nc


# ----------------------------------------------------------------------------
# Entry point
# ----------------------------------------------------------------------------

def _get(edge_index):
    key = (hashlib.sha1(np.ascontiguousarray(edge_index).tobytes()).hexdigest(), _NCONV, _USE_AR, _USE_AG)
    if key not in _CACHE:
        B, padn, ppos, srci, dstf, coww = _preprocess(edge_index)
        nc = _build(B, padn)
        _CACHE[key] = (nc, B, padn, ppos, srci, dstf, coww)
    return _CACHE[key]


def _run(inputs, trace=False):
    from concourse.bass_utils import run_bass_kernel_spmd

    edge_index = np.asarray(inputs["edge_index"])
    nc, B, padn, ppos, srci, dstf, coww = _get(edge_index)

    x = np.asarray(inputs["x"], dtype=np.float32)
    xtab = np.zeros((padn, D), dtype=np.float32)
    xtab[ppos] = x
    wcat = np.concatenate(
        [np.asarray(inputs["W_mu"], np.float32),
         np.asarray(inputs["W_logvar"], np.float32)], axis=1)
    iotab = np.tile(np.arange(128, dtype=np.float32), (128, TPB))
    gb0 = np.stack([np.asarray(inputs["gamma0"], np.float32),
                    np.asarray(inputs["beta0"], np.float32)], axis=1)
    gb1 = np.stack([np.asarray(inputs["gamma1"], np.float32),
                    np.asarray(inputs["beta1"], np.float32)], axis=1)

    in_maps = []
    for c in range(NCORES):
        in_maps.append({
            "xtab": xtab,
            "srci": srci[c], "dstf": dstf[c], "coef": coww[c],
            "iotab": iotab,
            "w0": np.asarray(inputs["W0"], np.float32),
            "w1": np.asarray(inputs["W1"], np.float32),
            "wcat": wcat, "gb0": gb0, "gb1": gb1,
        })
    res = run_bass_kernel_spmd(nc, in_maps, core_ids=list(range(NCORES)),
                               trace=trace)
    z_all = np.concatenate([res.results[c]["z"] for c in range(NCORES)], axis=0)
    z = z_all[ppos]
    return (z[:, :DLAT].copy(), z[:, DLAT:].copy()), res


def kernel(**inputs):
    (z_mean, z_log_std), _ = _run(inputs, trace=False)
    return (z_mean, z_log_std)


def kernel_traced(**inputs):
    """Like kernel() but returns (outputs, exec_time_ns) using NTFF tracing."""
    outs, res = _run(inputs, trace=True)
    return outs, res.exec_time_ns


# revision 6
# speedup vs baseline: 1.1229x; 1.1229x over previous
"""GCN encoder (3-layer: GCNConv+BN+ReLU ×2, then GCNConv to [mu|logvar]) on 8
Trainium2 NeuronCores via Bass/Tile.

Strategy (dst-sharded message passing):
- Nodes are sharded 8 ways by destination. Each core owns a contiguous dst
  range and processes only the edges pointing into it (plus self-loops,
  appended as ordinary edges with coef = dinv^2).
- Host preprocessing packs each core's edges into dst-blocks (<=128 dst nodes,
  <= TPB*128 edge slots each, zero-padded) so one static program serves all 8
  cores. Node ids are remapped into a padded global layout so gather tables
  and output shards line up with block boundaries.
- Per edge-tile (128 edges): a 128-row indirect DMA gathers src feature rows
  from the (replicated) table; DVE builds a one-hot selection matrix weighted
  by the edge coefficients (iota compare x coef); TensorE accumulates
  sel^T-weighted rows into a feature-major PSUM block: agg[feat, dst].
- Per block: the layer weight matmul runs on the aggregated block (linearity:
  S@(hW) == (S@h)@W), BN partial sums accumulate on DVE.
- BatchNorm: [128,2] AllReduce of (sum, sumsq) across cores; scale/shift and
  ReLU are fused into one ScalarE activation over the feature-major shard.
- Halo exchange: shards are PE-transposed to row-major and AllGathered into
  each core's next-layer gather table.
- Layer 3 uses W_cat = [W_mu | W_logvar] so both outputs ride one aggregation.
"""

import hashlib
import numpy as np

N = 50000
E = 800000
D = 128
DLAT = 64
EPS = 1e-5
NCORES = 8
NSHARD = N // NCORES          # 6250
TPB = 16                      # edge tiles per dst block (16*128 = 2048 slots)

_CACHE = {}
_NCONV = 3        # debug: number of convs to run
_USE_AR = True    # debug: BN AllReduce on/off (off -> local stats)
_USE_AG = True    # debug: AllGather on/off (off -> next conv gathers garbage)


# ----------------------------------------------------------------------------
# Host-side preprocessing
# ----------------------------------------------------------------------------

def _preprocess(edge_index):
    src = np.asarray(edge_index[0], dtype=np.int64)
    dst = np.asarray(edge_index[1], dtype=np.int64)
    deg = np.bincount(dst, minlength=N).astype(np.float32) + 1.0
    dinv = (1.0 / np.sqrt(deg)).astype(np.float32)

    # self-loops are handled by a sequential per-block path, not as edges
    src_a, dst_a = src, dst
    coef_a = dinv[src_a] * dinv[dst_a]

    cap = TPB * 128
    per_core = []
    max_blocks = 0
    for c in range(NCORES):
        lo, hi = c * NSHARD, (c + 1) * NSHARD
        m = (dst_a >= lo) & (dst_a < hi)
        s_c, d_c, f_c = src_a[m], dst_a[m] - lo, coef_a[m]
        order = np.argsort(d_c, kind="stable")
        s_c, d_c, f_c = s_c[order], d_c[order], f_c[order]
        cnt = np.bincount(d_c, minlength=NSHARD)
        # greedy block packing: <=128 dsts and <=cap edges per block
        blocks = []  # (dst_start, dst_end, edge_start, edge_end)
        v = 0
        epos = 0
        while v < NSHARD:
            v0, e0, ecnt = v, epos, 0
            while v < NSHARD and (v - v0) < 128 and ecnt + cnt[v] <= cap:
                ecnt += cnt[v]
                v += 1
            assert v > v0, f"node {v} degree {cnt[v]} exceeds capacity {cap}"
            epos += ecnt
            blocks.append((v0, v, e0, epos))
        per_core.append((s_c, d_c, f_c, blocks))
        max_blocks = max(max_blocks, len(blocks))

    B = max_blocks
    padn = NCORES * B * 128

    # padded global position of each node
    ppos = np.zeros(N, dtype=np.int64)
    for c in range(NCORES):
        _, _, _, blocks = per_core[c]
        for b, (v0, v1, _, _) in enumerate(blocks):
            ppos[c * NSHARD + v0:c * NSHARD + v1] = (
                c * B * 128 + b * 128 + np.arange(v1 - v0)
            )

    # packed per-core device arrays
    srci = np.zeros((NCORES, 128, B * TPB), dtype=np.int32)
    dstf = np.zeros((NCORES, 128, B * TPB), dtype=np.float32)
    coww = np.zeros((NCORES, 128, B * TPB), dtype=np.float32)
    for c in range(NCORES):
        s_c, d_c, f_c, blocks = per_core[c]
        sp = np.zeros(B * cap, dtype=np.int64)
        dp = np.zeros(B * cap, dtype=np.float32)
        fp = np.zeros(B * cap, dtype=np.float32)
        for b, (v0, v1, e0, e1) in enumerate(blocks):
            n = e1 - e0
            sp[b * cap:b * cap + n] = ppos[s_c[e0:e1]]
            dp[b * cap:b * cap + n] = (d_c[e0:e1] - v0).astype(np.float32)
            fp[b * cap:b * cap + n] = f_c[e0:e1]
        # wrap: [B*TPB tiles, 128] -> [128, B*TPB]; tile k edge p at [p, k]
        srci[c] = sp.reshape(B * TPB, 128).T
        dstf[c] = dp.reshape(B * TPB, 128).T
        coww[c] = fp.reshape(B * TPB, 128).T

    # per-core self-loop coefficients dinv^2 laid out [128, B] (block-major)
    dinv2w = np.zeros((NCORES, 128, B), dtype=np.float32)
    for c in range(NCORES):
        _, _, _, blocks = per_core[c]
        for b, (v0, v1, _, _) in enumerate(blocks):
            dv = dinv[c * NSHARD + v0:c * NSHARD + v1]
            dinv2w[c, :v1 - v0, b] = dv * dv

    return B, padn, ppos, srci, dstf, coww, dinv2w


# ----------------------------------------------------------------------------
# Device program
# ----------------------------------------------------------------------------

def _build(B, padn):
    from concourse import bacc, mybir
    import concourse.bass as bass
    import concourse.tile as tile
    from concourse.masks import make_identity

    F32 = mybir.dt.float32
    NB = B * 128  # padded shard width

    nc = bacc.Bacc("TRN2", target_bir_lowering=False, debug=False,
                   num_devices=NCORES)
    xtab = nc.dram_tensor("xtab", [padn, D], F32, kind="ExternalInput").ap()
    srci = nc.dram_tensor("srci", [128, B * TPB], mybir.dt.int32, kind="ExternalInput").ap()
    dstf = nc.dram_tensor("dstf", [128, B * TPB], F32, kind="ExternalInput").ap()
    coef = nc.dram_tensor("coef", [128, B * TPB], F32, kind="ExternalInput").ap()
    iotab = nc.dram_tensor("iotab", [128, TPB * 128], F32, kind="ExternalInput").ap()
    w0 = nc.dram_tensor("w0", [D, D], F32, kind="ExternalInput").ap()
    w1 = nc.dram_tensor("w1", [D, D], F32, kind="ExternalInput").ap()
    wcat = nc.dram_tensor("wcat", [D, D], F32, kind="ExternalInput").ap()
    gb0 = nc.dram_tensor("gb0", [128, 2], F32, kind="ExternalInput").ap()  # gamma0|beta0
    gb1 = nc.dram_tensor("gb1", [128, 2], F32, kind="ExternalInput").ap()
    xrm = nc.dram_tensor("xrm", [NB, D], F32, kind="ExternalInput").ap()
    dinv2 = nc.dram_tensor("dinv2", [128, B], F32, kind="ExternalInput").ap()
    zout = nc.dram_tensor("z", [NB, D], F32, kind="ExternalOutput").ap()

    htab1 = nc.dram_tensor("htab1", [padn, D], F32, addr_space="Shared")
    htab2 = nc.dram_tensor("htab2", [padn, D], F32, addr_space="Shared")
    hrm1 = nc.dram_tensor("hrm1", [NB, D], F32)
    hrm2 = nc.dram_tensor("hrm2", [NB, D], F32)
    bnin = [nc.dram_tensor(f"bnin{k}", [128, 2], F32) for k in range(2)]
    bnout = [nc.dram_tensor(f"bnout{k}", [128, 2], F32) for k in range(2)]

    with tile.TileContext(nc) as tc:
        with (
            tc.tile_pool(name="const", bufs=1) as constp,
            tc.tile_pool(name="gath", bufs=12) as gpool,
            tc.tile_pool(name="selp", bufs=3) as selp,
            tc.tile_pool(name="work", bufs=4) as wpool,
            tc.tile_pool(name="hacc", bufs=1) as haccp,
            tc.tile_pool(name="psA", bufs=2, space="PSUM") as psA,
            tc.tile_pool(name="psB", bufs=2, space="PSUM") as psB,
            tc.tile_pool(name="psT", bufs=2, space="PSUM") as psT,
        ):
            idx_t = constp.tile([128, B * TPB], mybir.dt.int32)
            dst_t = constp.tile([128, B * TPB], F32)
            cof_t = constp.tile([128, B * TPB], F32)
            iota_t = constp.tile([128, TPB, 128], F32)
            ident = constp.tile([128, 128], F32)
            w_t = [constp.tile([D, D], F32, name=f"w_t{k}", tag=f"w{k}") for k in range(3)]
            gb_t = [constp.tile([128, 2], F32, name=f"gb_t{k}", tag=f"gb{k}") for k in range(2)]
            dv2_t = constp.tile([128, B], F32)
            nc.sync.dma_start(out=idx_t[:], in_=srci[:])
            nc.sync.dma_start(out=dst_t[:], in_=dstf[:])
            nc.sync.dma_start(out=cof_t[:], in_=coef[:])
            nc.sync.dma_start(out=iota_t[:], in_=iotab[:].rearrange("p (t f) -> p t f", t=TPB))
            nc.sync.dma_start(out=w_t[0][:], in_=w0[:])
            nc.sync.dma_start(out=w_t[1][:], in_=w1[:])
            nc.sync.dma_start(out=w_t[2][:], in_=wcat[:])
            nc.sync.dma_start(out=gb_t[0][:], in_=gb0[:])
            nc.sync.dma_start(out=gb_t[1][:], in_=gb1[:])
            nc.sync.dma_start(out=dv2_t[:], in_=dinv2[:])
            make_identity(nc, ident[:])

            h_shard = haccp.tile([128, NB], F32)
            ssum = haccp.tile([128, 1], F32)
            ssq = haccp.tile([128, 1], F32)

            tabs = [xtab, htab1.ap(), htab2.ap()]
            selfsrc = [xrm, hrm1.ap(), hrm2.ap()]
            hrms = [hrm1, hrm2]
            htabs = [htab1, htab2]

            for k in range(_NCONV):
                tab = tabs[k]
                if k < 2:
                    nc.vector.memset(ssum[:], 0.0)
                    nc.vector.memset(ssq[:], 0.0)
                for b in range(B):
                    # one-hot (coef-weighted) selection for the whole block
                    sel = selp.tile([128, TPB, 128], F32)
                    c0 = b * TPB
                    nc.vector.tensor_tensor(
                        out=sel[:],
                        in0=dst_t[:, c0:c0 + TPB].to_broadcast([128, TPB, 128]),
                        in1=iota_t[:], op=mybir.AluOpType.is_equal,
                    )
                    nc.vector.tensor_tensor(
                        out=sel[:], in0=sel[:],
                        in1=cof_t[:, c0:c0 + TPB].to_broadcast([128, TPB, 128]),
                        op=mybir.AluOpType.mult,
                    )
                    agg_ps = psA.tile([128, 128], F32, space="PSUM")
                    for t in range(TPB):
                        kk = c0 + t
                        g = gpool.tile([128, D], F32)
                        nc.gpsimd.indirect_dma_start(
                            out=g[:], out_offset=None, in_=tab,
                            in_offset=bass.IndirectOffsetOnAxis(
                                ap=idx_t[:, kk:kk + 1], axis=0),
                        )
                        # agg[feat, dst] += g[e, feat]^T @ sel[e, dst]
                        nc.tensor.matmul(out=agg_ps[:], lhsT=g[:],
                                         rhs=sel[:, t, :],
                                         start=(t == 0), stop=False)
                    # self-loop: agg[:, d] += dinv2[d] * h_prev[d, :]^T  via
                    # a transpose-matmul (identity rhs) accumulated into PSUM
                    srows = gpool.tile([128, D], F32, tag="srows")
                    nc.sync.dma_start(out=srows[:],
                                      in_=selfsrc[k][b * 128:(b + 1) * 128, :])
                    sscl = wpool.tile([128, D], F32, tag="sscl")
                    nc.scalar.activation(out=sscl[:], in_=srows[:],
                                         func=mybir.ActivationFunctionType.Copy,
                                         scale=dv2_t[:, b:b + 1])
                    nc.tensor.matmul(out=agg_ps[:], lhsT=sscl[:], rhs=ident[:],
                                     start=False, stop=True)
                    aggs = wpool.tile([128, 128], F32, tag="aggs")
                    nc.vector.tensor_copy(out=aggs[:], in_=agg_ps[:])
                    # h[fo, dst] = W[fi, fo]^T @ aggs[fi, dst]
                    h_ps = psB.tile([128, 128], F32, space="PSUM")
                    nc.tensor.matmul(out=h_ps[:], lhsT=w_t[k][:], rhs=aggs[:],
                                     start=True, stop=True)
                    h_sl = h_shard[:, b * 128:(b + 1) * 128]
                    nc.vector.tensor_copy(out=h_sl, in_=h_ps[:])
                    if k < 2:
                        rsum = wpool.tile([128, 1], F32, tag="rsum")
                        nc.vector.tensor_reduce(out=rsum[:], in_=h_ps[:],
                                                axis=mybir.AxisListType.X,
                                                op=mybir.AluOpType.add)
                        nc.vector.tensor_add(out=ssum[:], in0=ssum[:], in1=rsum[:])
                        sq = wpool.tile([128, 128], F32, tag="sq")
                        nc.vector.tensor_tensor(out=sq[:], in0=h_sl, in1=h_sl,
                                                op=mybir.AluOpType.mult)
                        nc.vector.tensor_reduce(out=rsum[:], in_=sq[:],
                                                axis=mybir.AxisListType.X,
                                                op=mybir.AluOpType.add)
                        nc.vector.tensor_add(out=ssq[:], in0=ssq[:], in1=rsum[:])

                if k < 2:
                    # cross-core BN stats
                    stl = wpool.tile([128, 2], F32, tag="stl")
                    nc.vector.tensor_copy(out=stl[:, 0:1], in_=ssum[:])
                    nc.vector.tensor_copy(out=stl[:, 1:2], in_=ssq[:])
                    nc.sync.dma_start(out=bnin[k].ap(), in_=stl[:])
                    stg = wpool.tile([128, 2], F32, tag="stg")
                    if _USE_AR:
                        nc.gpsimd.collective_compute(
                            "AllReduce", mybir.AluOpType.add,
                            replica_groups=[list(range(NCORES))],
                            ins=[bnin[k].ap().opt()], outs=[bnout[k].ap().opt()],
                        )
                        nc.sync.dma_start(out=stg[:], in_=bnout[k].ap())
                    else:
                        nc.sync.dma_start(out=stg[:], in_=bnin[k].ap())
                    # a = gamma*rsqrt(var+eps); b = beta - mean*a
                    mean = wpool.tile([128, 1], F32, tag="mean")
                    var = wpool.tile([128, 1], F32, tag="var")
                    a_t = wpool.tile([128, 1], F32, tag="a_t")
                    b_t = wpool.tile([128, 1], F32, tag="b_t")
                    inv_n = 1.0 / float(N)
                    nc.vector.tensor_scalar_mul(out=mean[:], in0=stg[:, 0:1], scalar1=inv_n)
                    nc.vector.tensor_scalar_mul(out=var[:], in0=stg[:, 1:2], scalar1=inv_n)
                    msq = wpool.tile([128, 1], F32, tag="msq")
                    nc.vector.tensor_tensor(out=msq[:], in0=mean[:], in1=mean[:],
                                            op=mybir.AluOpType.mult)
                    nc.vector.tensor_tensor(out=var[:], in0=var[:], in1=msq[:],
                                            op=mybir.AluOpType.subtract)
                    nc.vector.tensor_scalar(out=var[:], in0=var[:], scalar1=float(EPS),
                                            scalar2=None, op0=mybir.AluOpType.add)
                    nc.scalar.activation(out=a_t[:], in_=var[:],
                                         func=mybir.ActivationFunctionType.Sqrt)
                    nc.vector.reciprocal(out=a_t[:], in_=a_t[:])
                    nc.vector.tensor_tensor(out=a_t[:], in0=a_t[:],
                                            in1=gb_t[k][:, 0:1],
                                            op=mybir.AluOpType.mult)
                    nc.vector.tensor_tensor(out=b_t[:], in0=mean[:], in1=a_t[:],
                                            op=mybir.AluOpType.mult)
                    nc.vector.tensor_tensor(out=b_t[:], in0=gb_t[k][:, 1:2],
                                            in1=b_t[:],
                                            op=mybir.AluOpType.subtract)
                    # h = relu(a*h + b), fused on ScalarE
                    nc.scalar.activation(out=h_shard[:], in_=h_shard[:],
                                         func=mybir.ActivationFunctionType.Relu,
                                         scale=a_t[:], bias=b_t[:])

                # export shard row-major (+ halo exchange for k<2)
                is_last = (k == _NCONV - 1)
                dst_dram = None if is_last else hrms[k]
                for b in range(B):
                    tp = psT.tile([128, 128], F32, space="PSUM")
                    nc.tensor.transpose(out=tp[:],
                                        in_=h_shard[:, b * 128:(b + 1) * 128],
                                        identity=ident[:])
                    rm = wpool.tile([128, 128], F32, tag="rm")
                    nc.vector.tensor_copy(out=rm[:], in_=tp[:])
                    if not is_last:
                        nc.sync.dma_start(out=dst_dram.ap()[b * 128:(b + 1) * 128, :],
                                          in_=rm[:])
                    else:
                        nc.sync.dma_start(out=zout[b * 128:(b + 1) * 128, :],
                                          in_=rm[:])
                if k < 2 and k < _NCONV - 1:
                    if _USE_AG:
                        nc.gpsimd.collective_compute(
                            "AllGather", mybir.AluOpType.bypass,
                            replica_groups=[list(range(NCORES))],
                            ins=[hrms[k].ap().opt()], outs=[htabs[k].ap().opt()],
                        )
                    else:
                        nc.sync.dma_start(out=htabs[k].ap()[0:NB, :], in_=hrms[k].ap())

    nc.compile()
    return nc


# ----------------------------------------------------------------------------
# Entry point
# ----------------------------------------------------------------------------

def _get(edge_index):
    key = (hashlib.sha1(np.ascontiguousarray(edge_index).tobytes()).hexdigest(), _NCONV, _USE_AR, _USE_AG)
    if key not in _CACHE:
        B, padn, ppos, srci, dstf, coww, dinv2w = _preprocess(edge_index)
        nc = _build(B, padn)
        _CACHE[key] = (nc, B, padn, ppos, srci, dstf, coww, dinv2w)
    return _CACHE[key]


def _run(inputs, trace=False):
    from concourse.bass_utils import run_bass_kernel_spmd

    edge_index = np.asarray(inputs["edge_index"])
    nc, B, padn, ppos, srci, dstf, coww, dinv2w = _get(edge_index)

    x = np.asarray(inputs["x"], dtype=np.float32)
    xtab = np.zeros((padn, D), dtype=np.float32)
    xtab[ppos] = x
    wcat = np.concatenate(
        [np.asarray(inputs["W_mu"], np.float32),
         np.asarray(inputs["W_logvar"], np.float32)], axis=1)
    iotab = np.tile(np.arange(128, dtype=np.float32), (128, TPB))
    gb0 = np.stack([np.asarray(inputs["gamma0"], np.float32),
                    np.asarray(inputs["beta0"], np.float32)], axis=1)
    gb1 = np.stack([np.asarray(inputs["gamma1"], np.float32),
                    np.asarray(inputs["beta1"], np.float32)], axis=1)

    in_maps = []
    for c in range(NCORES):
        NB = B * 128
        in_maps.append({
            "xtab": xtab,
            "xrm": xtab[c * NB:(c + 1) * NB],
            "dinv2": dinv2w[c],
            "srci": srci[c], "dstf": dstf[c], "coef": coww[c],
            "iotab": iotab,
            "w0": np.asarray(inputs["W0"], np.float32),
            "w1": np.asarray(inputs["W1"], np.float32),
            "wcat": wcat, "gb0": gb0, "gb1": gb1,
        })
    res = run_bass_kernel_spmd(nc, in_maps, core_ids=list(range(NCORES)),
                               trace=trace)
    z_all = np.concatenate([res.results[c]["z"] for c in range(NCORES)], axis=0)
    z = z_all[ppos]
    return (z[:, :DLAT].copy(), z[:, DLAT:].copy()), res


def kernel(**inputs):
    (z_mean, z_log_std), _ = _run(inputs, trace=False)
    return (z_mean, z_log_std)


def kernel_traced(**inputs):
    """Like kernel() but returns (outputs, exec_time_ns) using NTFF tracing."""
    outs, res = _run(inputs, trace=True)
    return outs, res.exec_time_ns


# revision 7
# speedup vs baseline: 1.1246x; 1.0015x over previous
"""GCN encoder (3-layer: GCNConv+BN+ReLU ×2, then GCNConv to [mu|logvar]) on 8
Trainium2 NeuronCores via Bass/Tile.

Strategy (dst-sharded message passing):
- Nodes are sharded 8 ways by destination. Each core owns a contiguous dst
  range and processes only the edges pointing into it (plus self-loops,
  appended as ordinary edges with coef = dinv^2).
- Host preprocessing packs each core's edges into dst-blocks (<=128 dst nodes,
  <= TPB*128 edge slots each, zero-padded) so one static program serves all 8
  cores. Node ids are remapped into a padded global layout so gather tables
  and output shards line up with block boundaries.
- Per edge-tile (128 edges): a 128-row indirect DMA gathers src feature rows
  from the (replicated) table; DVE builds a one-hot selection matrix weighted
  by the edge coefficients (iota compare x coef); TensorE accumulates
  sel^T-weighted rows into a feature-major PSUM block: agg[feat, dst].
- Per block: the layer weight matmul runs on the aggregated block (linearity:
  S@(hW) == (S@h)@W), BN partial sums accumulate on DVE.
- BatchNorm: [128,2] AllReduce of (sum, sumsq) across cores; scale/shift and
  ReLU are fused into one ScalarE activation over the feature-major shard.
- Halo exchange: shards are PE-transposed to row-major and AllGathered into
  each core's next-layer gather table.
- Layer 3 uses W_cat = [W_mu | W_logvar] so both outputs ride one aggregation.
"""

import hashlib
import numpy as np

N = 50000
E = 800000
D = 128
DLAT = 64
EPS = 1e-5
NCORES = 8
NSHARD = N // NCORES          # 6250
TPB = 16                      # edge tiles per dst block (16*128 = 2048 slots)

_CACHE = {}
_NCONV = 3        # debug: number of convs to run
_USE_AR = True    # debug: BN AllReduce on/off (off -> local stats)
_USE_AG = True    # debug: AllGather on/off (off -> next conv gathers garbage)


# ----------------------------------------------------------------------------
# Host-side preprocessing
# ----------------------------------------------------------------------------

def _preprocess(edge_index):
    src = np.asarray(edge_index[0], dtype=np.int64)
    dst = np.asarray(edge_index[1], dtype=np.int64)
    deg = np.bincount(dst, minlength=N).astype(np.float32) + 1.0
    dinv = (1.0 / np.sqrt(deg)).astype(np.float32)

    # self-loops are handled by a sequential per-block path, not as edges
    src_a, dst_a = src, dst
    coef_a = dinv[src_a] * dinv[dst_a]

    cap = TPB * 128
    per_core = []
    max_blocks = 0
    for c in range(NCORES):
        lo, hi = c * NSHARD, (c + 1) * NSHARD
        m = (dst_a >= lo) & (dst_a < hi)
        s_c, d_c, f_c = src_a[m], dst_a[m] - lo, coef_a[m]
        order = np.argsort(d_c, kind="stable")
        s_c, d_c, f_c = s_c[order], d_c[order], f_c[order]
        cnt = np.bincount(d_c, minlength=NSHARD)
        # greedy block packing: <=128 dsts and <=cap edges per block
        blocks = []  # (dst_start, dst_end, edge_start, edge_end)
        v = 0
        epos = 0
        while v < NSHARD:
            v0, e0, ecnt = v, epos, 0
            while v < NSHARD and (v - v0) < 128 and ecnt + cnt[v] <= cap:
                ecnt += cnt[v]
                v += 1
            assert v > v0, f"node {v} degree {cnt[v]} exceeds capacity {cap}"
            epos += ecnt
            blocks.append((v0, v, e0, epos))
        per_core.append((s_c, d_c, f_c, blocks))
        max_blocks = max(max_blocks, len(blocks))

    B = max_blocks
    padn = NCORES * B * 128

    # padded global position of each node
    ppos = np.zeros(N, dtype=np.int64)
    for c in range(NCORES):
        _, _, _, blocks = per_core[c]
        for b, (v0, v1, _, _) in enumerate(blocks):
            ppos[c * NSHARD + v0:c * NSHARD + v1] = (
                c * B * 128 + b * 128 + np.arange(v1 - v0)
            )

    # packed per-core device arrays
    srci = np.zeros((NCORES, 128, B * TPB), dtype=np.int32)
    dstf = np.zeros((NCORES, 128, B * TPB), dtype=np.float32)
    coww = np.zeros((NCORES, 128, B * TPB), dtype=np.float32)
    for c in range(NCORES):
        s_c, d_c, f_c, blocks = per_core[c]
        sp = np.zeros(B * cap, dtype=np.int64)
        dp = np.zeros(B * cap, dtype=np.float32)
        fp = np.zeros(B * cap, dtype=np.float32)
        for b, (v0, v1, e0, e1) in enumerate(blocks):
            n = e1 - e0
            sp[b * cap:b * cap + n] = ppos[s_c[e0:e1]]
            dp[b * cap:b * cap + n] = (d_c[e0:e1] - v0).astype(np.float32)
            fp[b * cap:b * cap + n] = f_c[e0:e1]
        # wrap: [B*TPB tiles, 128] -> [128, B*TPB]; tile k edge p at [p, k]
        srci[c] = sp.reshape(B * TPB, 128).T
        dstf[c] = dp.reshape(B * TPB, 128).T
        coww[c] = fp.reshape(B * TPB, 128).T

    # per-core self-loop coefficients dinv^2 laid out [128, B] (block-major)
    dinv2w = np.zeros((NCORES, 128, B), dtype=np.float32)
    for c in range(NCORES):
        _, _, _, blocks = per_core[c]
        for b, (v0, v1, _, _) in enumerate(blocks):
            dv = dinv[c * NSHARD + v0:c * NSHARD + v1]
            dinv2w[c, :v1 - v0, b] = dv * dv

    return B, padn, ppos, srci, dstf, coww, dinv2w


# ----------------------------------------------------------------------------
# Device program
# ----------------------------------------------------------------------------

def _build(B, padn):
    from concourse import bacc, mybir
    import concourse.bass as bass
    import concourse.tile as tile
    from concourse.masks import make_identity

    F32 = mybir.dt.float32
    NB = B * 128  # padded shard width

    nc = bacc.Bacc("TRN2", target_bir_lowering=False, debug=False,
                   num_devices=NCORES)
    xtab = nc.dram_tensor("xtab", [padn, D], F32, kind="ExternalInput").ap()
    srci = nc.dram_tensor("srci", [128, B * TPB], mybir.dt.int32, kind="ExternalInput").ap()
    dstf = nc.dram_tensor("dstf", [128, B * TPB], F32, kind="ExternalInput").ap()
    coef = nc.dram_tensor("coef", [128, B * TPB], F32, kind="ExternalInput").ap()
    iotab = nc.dram_tensor("iotab", [128, TPB * 128], F32, kind="ExternalInput").ap()
    w0 = nc.dram_tensor("w0", [D, D], F32, kind="ExternalInput").ap()
    w1 = nc.dram_tensor("w1", [D, D], F32, kind="ExternalInput").ap()
    wcat = nc.dram_tensor("wcat", [D, D], F32, kind="ExternalInput").ap()
    gb0 = nc.dram_tensor("gb0", [128, 2], F32, kind="ExternalInput").ap()  # gamma0|beta0
    gb1 = nc.dram_tensor("gb1", [128, 2], F32, kind="ExternalInput").ap()
    xrm = nc.dram_tensor("xrm", [NB, D], F32, kind="ExternalInput").ap()
    dinv2 = nc.dram_tensor("dinv2", [128, B], F32, kind="ExternalInput").ap()
    zout = nc.dram_tensor("z", [NB, D], F32, kind="ExternalOutput").ap()

    htab1 = nc.dram_tensor("htab1", [padn, D], F32, addr_space="Shared")
    htab2 = nc.dram_tensor("htab2", [padn, D], F32, addr_space="Shared")
    hrm1 = nc.dram_tensor("hrm1", [NB, D], F32)
    hrm2 = nc.dram_tensor("hrm2", [NB, D], F32)
    bnin = [nc.dram_tensor(f"bnin{k}", [128, 2], F32) for k in range(2)]
    bnout = [nc.dram_tensor(f"bnout{k}", [128, 2], F32) for k in range(2)]

    with tile.TileContext(nc) as tc:
        with (
            tc.tile_pool(name="const", bufs=1) as constp,
            tc.tile_pool(name="gath", bufs=12) as gpool,
            tc.tile_pool(name="selp", bufs=3) as selp,
            tc.tile_pool(name="work", bufs=4) as wpool,
            tc.tile_pool(name="hacc", bufs=1) as haccp,
            tc.tile_pool(name="psA", bufs=2, space="PSUM") as psA,
            tc.tile_pool(name="psB", bufs=2, space="PSUM") as psB,
            tc.tile_pool(name="psT", bufs=2, space="PSUM") as psT,
        ):
            idx_t = constp.tile([128, B * TPB], mybir.dt.int32)
            dst_t = constp.tile([128, B * TPB], F32)
            cof_t = constp.tile([128, B * TPB], F32)
            iota_t = constp.tile([128, TPB, 128], F32)
            ident = constp.tile([128, 128], F32)
            w_t = [constp.tile([D, D], F32, name=f"w_t{k}", tag=f"w{k}") for k in range(3)]
            gb_t = [constp.tile([128, 2], F32, name=f"gb_t{k}", tag=f"gb{k}") for k in range(2)]
            dv2_t = constp.tile([128, B], F32)
            nc.sync.dma_start(out=idx_t[:], in_=srci[:])
            nc.sync.dma_start(out=dst_t[:], in_=dstf[:])
            nc.sync.dma_start(out=cof_t[:], in_=coef[:])
            nc.sync.dma_start(out=iota_t[:], in_=iotab[:].rearrange("p (t f) -> p t f", t=TPB))
            nc.sync.dma_start(out=w_t[0][:], in_=w0[:])
            nc.sync.dma_start(out=w_t[1][:], in_=w1[:])
            nc.sync.dma_start(out=w_t[2][:], in_=wcat[:])
            nc.sync.dma_start(out=gb_t[0][:], in_=gb0[:])
            nc.sync.dma_start(out=gb_t[1][:], in_=gb1[:])
            nc.sync.dma_start(out=dv2_t[:], in_=dinv2[:])
            make_identity(nc, ident[:])

            h_shard = haccp.tile([128, NB], F32)
            ssum = haccp.tile([128, 1], F32)
            ssq = haccp.tile([128, 1], F32)

            tabs = [xtab, htab1.ap(), htab2.ap()]
            selfsrc = [xrm, hrm1.ap(), hrm2.ap()]
            hrms = [hrm1, hrm2]
            htabs = [htab1, htab2]

            for k in range(_NCONV):
                tab = tabs[k]
                if k < 2:
                    nc.vector.memset(ssum[:], 0.0)
                    nc.vector.memset(ssq[:], 0.0)
                for b in range(B):
                    # one-hot (coef-weighted) selection for the whole block
                    sel = selp.tile([128, TPB, 128], F32)
                    c0 = b * TPB
                    nc.vector.tensor_tensor(
                        out=sel[:],
                        in0=dst_t[:, c0:c0 + TPB].to_broadcast([128, TPB, 128]),
                        in1=iota_t[:], op=mybir.AluOpType.is_equal,
                    )
                    nc.vector.tensor_tensor(
                        out=sel[:], in0=sel[:],
                        in1=cof_t[:, c0:c0 + TPB].to_broadcast([128, TPB, 128]),
                        op=mybir.AluOpType.mult,
                    )
                    agg_ps = psA.tile([128, 128], F32, space="PSUM")
                    for t in range(TPB):
                        kk = c0 + t
                        g = gpool.tile([128, D], F32)
                        nc.gpsimd.indirect_dma_start(
                            out=g[:], out_offset=None, in_=tab,
                            in_offset=bass.IndirectOffsetOnAxis(
                                ap=idx_t[:, kk:kk + 1], axis=0),
                        )
                        # agg[feat, dst] += g[e, feat]^T @ sel[e, dst]
                        nc.tensor.matmul(out=agg_ps[:], lhsT=g[:],
                                         rhs=sel[:, t, :],
                                         start=(t == 0), stop=False)
                    # self-loop: agg[:, d] += dinv2[d] * h_prev[d, :]^T  via
                    # a transpose-matmul (identity rhs) accumulated into PSUM
                    srows = gpool.tile([128, D], F32, tag="srows")
                    nc.sync.dma_start(out=srows[:],
                                      in_=selfsrc[k][b * 128:(b + 1) * 128, :])
                    sscl = wpool.tile([128, D], F32, tag="sscl")
                    nc.scalar.activation(out=sscl[:], in_=srows[:],
                                         func=mybir.ActivationFunctionType.Copy,
                                         scale=dv2_t[:, b:b + 1])
                    nc.tensor.matmul(out=agg_ps[:], lhsT=sscl[:], rhs=ident[:],
                                     start=False, stop=True)
                    aggs = wpool.tile([128, 128], F32, tag="aggs")
                    nc.vector.tensor_copy(out=aggs[:], in_=agg_ps[:])
                    # h[fo, dst] = W[fi, fo]^T @ aggs[fi, dst]
                    h_ps = psB.tile([128, 128], F32, space="PSUM")
                    nc.tensor.matmul(out=h_ps[:], lhsT=w_t[k][:], rhs=aggs[:],
                                     start=True, stop=True)
                    h_sl = h_shard[:, b * 128:(b + 1) * 128]
                    nc.vector.tensor_copy(out=h_sl, in_=h_ps[:])
                    if k == _NCONV - 1:
                        tp = psT.tile([128, 128], F32, space="PSUM", name="tp2", tag="tp2")
                        nc.tensor.transpose(out=tp[:], in_=h_sl, identity=ident[:])
                        rm = wpool.tile([128, 128], F32, name="rm2", tag="rm2")
                        nc.vector.tensor_copy(out=rm[:], in_=tp[:])
                        nc.sync.dma_start(out=zout[b * 128:(b + 1) * 128, :], in_=rm[:])
                    if k < 2:
                        rsum = wpool.tile([128, 1], F32, tag="rsum")
                        nc.vector.tensor_reduce(out=rsum[:], in_=h_ps[:],
                                                axis=mybir.AxisListType.X,
                                                op=mybir.AluOpType.add)
                        nc.vector.tensor_add(out=ssum[:], in0=ssum[:], in1=rsum[:])
                        sq = wpool.tile([128, 128], F32, tag="sq")
                        nc.vector.tensor_tensor(out=sq[:], in0=h_sl, in1=h_sl,
                                                op=mybir.AluOpType.mult)
                        nc.vector.tensor_reduce(out=rsum[:], in_=sq[:],
                                                axis=mybir.AxisListType.X,
                                                op=mybir.AluOpType.add)
                        nc.vector.tensor_add(out=ssq[:], in0=ssq[:], in1=rsum[:])

                if k < 2:
                    # cross-core BN stats
                    stl = wpool.tile([128, 2], F32, tag="stl")
                    nc.vector.tensor_copy(out=stl[:, 0:1], in_=ssum[:])
                    nc.vector.tensor_copy(out=stl[:, 1:2], in_=ssq[:])
                    nc.sync.dma_start(out=bnin[k].ap(), in_=stl[:])
                    stg = wpool.tile([128, 2], F32, tag="stg")
                    if _USE_AR:
                        nc.gpsimd.collective_compute(
                            "AllReduce", mybir.AluOpType.add,
                            replica_groups=[list(range(NCORES))],
                            ins=[bnin[k].ap().opt()], outs=[bnout[k].ap().opt()],
                        )
                        nc.sync.dma_start(out=stg[:], in_=bnout[k].ap())
                    else:
                        nc.sync.dma_start(out=stg[:], in_=bnin[k].ap())
                    # a = gamma*rsqrt(var+eps); b = beta - mean*a
                    mean = wpool.tile([128, 1], F32, tag="mean")
                    var = wpool.tile([128, 1], F32, tag="var")
                    a_t = wpool.tile([128, 1], F32, tag="a_t")
                    b_t = wpool.tile([128, 1], F32, tag="b_t")
                    inv_n = 1.0 / float(N)
                    nc.vector.tensor_scalar_mul(out=mean[:], in0=stg[:, 0:1], scalar1=inv_n)
                    nc.vector.tensor_scalar_mul(out=var[:], in0=stg[:, 1:2], scalar1=inv_n)
                    msq = wpool.tile([128, 1], F32, tag="msq")
                    nc.vector.tensor_tensor(out=msq[:], in0=mean[:], in1=mean[:],
                                            op=mybir.AluOpType.mult)
                    nc.vector.tensor_tensor(out=var[:], in0=var[:], in1=msq[:],
                                            op=mybir.AluOpType.subtract)
                    nc.vector.tensor_scalar(out=var[:], in0=var[:], scalar1=float(EPS),
                                            scalar2=None, op0=mybir.AluOpType.add)
                    nc.scalar.activation(out=a_t[:], in_=var[:],
                                         func=mybir.ActivationFunctionType.Sqrt)
                    nc.vector.reciprocal(out=a_t[:], in_=a_t[:])
                    nc.vector.tensor_tensor(out=a_t[:], in0=a_t[:],
                                            in1=gb_t[k][:, 0:1],
                                            op=mybir.AluOpType.mult)
                    nc.vector.tensor_tensor(out=b_t[:], in0=mean[:], in1=a_t[:],
                                            op=mybir.AluOpType.mult)
                    nc.vector.tensor_tensor(out=b_t[:], in0=gb_t[k][:, 1:2],
                                            in1=b_t[:],
                                            op=mybir.AluOpType.subtract)
                    # h = relu(a*h + b), fused on ScalarE
                    nc.scalar.activation(out=h_shard[:], in_=h_shard[:],
                                         func=mybir.ActivationFunctionType.Relu,
                                         scale=a_t[:], bias=b_t[:])

                # export shard row-major for the halo exchange (k<2 only;
                # the last conv exported per-block above)
                if k < _NCONV - 1 and k < 2:
                    for b in range(B):
                        tp = psT.tile([128, 128], F32, space="PSUM")
                        nc.tensor.transpose(out=tp[:],
                                            in_=h_shard[:, b * 128:(b + 1) * 128],
                                            identity=ident[:])
                        rm = wpool.tile([128, 128], F32, tag="rm")
                        nc.vector.tensor_copy(out=rm[:], in_=tp[:])
                        nc.sync.dma_start(out=hrms[k].ap()[b * 128:(b + 1) * 128, :],
                                          in_=rm[:])
                if k < 2 and k < _NCONV - 1:
                    if _USE_AG:
                        nc.gpsimd.collective_compute(
                            "AllGather", mybir.AluOpType.bypass,
                            replica_groups=[list(range(NCORES))],
                            ins=[hrms[k].ap().opt()], outs=[htabs[k].ap().opt()],
                        )
                    else:
                        nc.sync.dma_start(out=htabs[k].ap()[0:NB, :], in_=hrms[k].ap())

    nc.compile()
    return nc


# ----------------------------------------------------------------------------
# Entry point
# ----------------------------------------------------------------------------

def _get(edge_index):
    key = (hashlib.sha1(np.ascontiguousarray(edge_index).tobytes()).hexdigest(), _NCONV, _USE_AR, _USE_AG)
    if key not in _CACHE:
        B, padn, ppos, srci, dstf, coww, dinv2w = _preprocess(edge_index)
        nc = _build(B, padn)
        _CACHE[key] = (nc, B, padn, ppos, srci, dstf, coww, dinv2w)
    return _CACHE[key]


def _run(inputs, trace=False):
    from concourse.bass_utils import run_bass_kernel_spmd

    edge_index = np.asarray(inputs["edge_index"])
    nc, B, padn, ppos, srci, dstf, coww, dinv2w = _get(edge_index)

    x = np.asarray(inputs["x"], dtype=np.float32)
    xtab = np.zeros((padn, D), dtype=np.float32)
    xtab[ppos] = x
    wcat = np.concatenate(
        [np.asarray(inputs["W_mu"], np.float32),
         np.asarray(inputs["W_logvar"], np.float32)], axis=1)
    iotab = np.tile(np.arange(128, dtype=np.float32), (128, TPB))
    gb0 = np.stack([np.asarray(inputs["gamma0"], np.float32),
                    np.asarray(inputs["beta0"], np.float32)], axis=1)
    gb1 = np.stack([np.asarray(inputs["gamma1"], np.float32),
                    np.asarray(inputs["beta1"], np.float32)], axis=1)

    in_maps = []
    for c in range(NCORES):
        NB = B * 128
        in_maps.append({
            "xtab": xtab,
            "xrm": xtab[c * NB:(c + 1) * NB],
            "dinv2": dinv2w[c],
            "srci": srci[c], "dstf": dstf[c], "coef": coww[c],
            "iotab": iotab,
            "w0": np.asarray(inputs["W0"], np.float32),
            "w1": np.asarray(inputs["W1"], np.float32),
            "wcat": wcat, "gb0": gb0, "gb1": gb1,
        })
    res = run_bass_kernel_spmd(nc, in_maps, core_ids=list(range(NCORES)),
                               trace=trace)
    z_all = np.concatenate([res.results[c]["z"] for c in range(NCORES)], axis=0)
    z = z_all[ppos]
    return (z[:, :DLAT].copy(), z[:, DLAT:].copy()), res


def kernel(**inputs):
    (z_mean, z_log_std), _ = _run(inputs, trace=False)
    return (z_mean, z_log_std)


def kernel_traced(**inputs):
    """Like kernel() but returns (outputs, exec_time_ns) using NTFF tracing."""
    outs, res = _run(inputs, trace=True)
    return outs, res.exec_time_ns
